# revision 10
# baseline (speedup 1.0000x reference)
"""Trainium2 Bass kernel for a 2-window local-attention layer.

Computation (see the module docstring of the harness reference):
  qkv = x @ qkv_w.T -> q,k,v  [B,H,N,D]
  scores = q k^T / sqrt(D); for ws in (9, 17): out += softmax(band-masked scores) @ v
  out = out/2 -> [B,N,C] @ proj_w.T + proj_b

Strategy: pure data parallelism over 8 cores.  The flattened (B*N = 4096)
token axis is cut into 8 shards of 512 tokens; each shard also loads an
8-token halo on both sides (zero padded at batch boundaries) so all keys and
values a query can attend to are local.  No collectives.

Per core everything is computed with the tensor engine:
  - qk^T in feature-major layout (features on partitions) via matmuls against
    pre-transposed weights (host passes W^T; q rows pre-scaled by 1/sqrt(D)),
  - v in token-major layout, with an extra all-ones column so that the
    PV matmuls emit the softmax denominators for free,
  - banded scores per 128-query block against its 144 keys, split into a
    128-key chunk and a 16-key tail chunk, in transposed [key, query] layout,
  - the band mask is pre-filled into PSUM with an identity matmul, so the
    exp comes straight off the scores matmul,
  - window-9 weights are the window-17 weights times a 0/1 inner-band mask,
  - softmax normalization happens on the PV *outputs* (queries on partitions,
    so the denominators are per-partition scalars),
  - attention output is transposed back with PE transposes and fed to the
    output projection; proj bias is matmul-broadcast into PSUM first.

Boundary handling: zero-padded halo keys produce score 0 -> exp 0+mask = 1
inside the band, so the host passes a per-query additive correction that
subtracts the padded-key count from each softmax denominator.  Padded values
are zero so PV numerators are unaffected.
"""

import os
import sys

import numpy as np

for _p in ("/opt/pypackages", "/opt/trn_rl_repo"):
    if _p not in sys.path:
        sys.path.insert(0, _p)

B, N, C = 2, 2048, 512
H, D = 8, 64
NCORES = 8
LOC = B * N // NCORES          # 512 tokens per core
HALO = 8
R = LOC + 2 * HALO             # 528 halo rows per core
CCH = C // 128                 # 4 contraction chunks
MASKV = -1.0e5                 # additive out-of-band mask (exp -> exactly 0)
SCALE = D ** -0.5

# stash of the last hardware run results (test.py reads exec_time_ns off this)
LAST_RESULTS = None


def _split_multi_waits(nc, mybir):
    """walrus codegen on this toolchain only encodes ONE sync-wait command per
    instruction; Tile freely emits several.  Hoist all but the last wait onto
    NoOp instructions inserted just before the offender (same engine, same
    block) — semantically identical, and each instruction ends up with <=1
    wait."""
    cnt = 0
    for fn in nc.m.functions:
        for blk in fn.blocks:
            out = []
            for inst in blk.instructions:
                si = inst.sync_info
                waits = list(si.on_wait) if si is not None and si.on_wait else []
                if len(waits) > 1:
                    for w in waits[:-1]:
                        cnt += 1
                        nop = mybir.InstNoOp(name=f"I-waitsplit-{cnt}",
                                             ins=[], outs=[])
                        nop.engine = inst.engine
                        nop.sync_info = mybir.SyncInfo(on_wait=[w], on_update=[])
                        out.append(nop)
                    si.on_wait = [waits[-1]]
                    inst.sync_info = si
                out.append(inst)
            blk.instructions = out


def build_module():
    import concourse.bass as bass
    import concourse.tile as tile
    import concourse.mybir as mybir
    from concourse.masks import make_identity

    f32 = mybir.dt.float32
    Exp = mybir.ActivationFunctionType.Exp
    is_ge = mybir.AluOpType.is_ge
    is_le = mybir.AluOpType.is_le

    nc = bass.Bass("TRN2", target_bir_lowering=False, debug=False)

    xT = nc.dram_tensor("xT", [128, CCH, R], f32, kind="ExternalInput").ap()
    wqkvT = nc.dram_tensor("wqkvT", [128, CCH, 3 * C], f32, kind="ExternalInput").ap()
    wprojT = nc.dram_tensor("wprojT", [128, CCH, C], f32, kind="ExternalInput").ap()
    pb = nc.dram_tensor("pb", [1, C], f32, kind="ExternalInput").ap()
    corr = nc.dram_tensor("corr", [128, 4, 2], f32, kind="ExternalInput").ap()
    y = nc.dram_tensor("y", [LOC, C], f32, kind="ExternalOutput").ap()

    with tile.TileContext(nc) as tc:
        with tc.tile_pool(name="const", bufs=1) as const, \
             tc.tile_pool(name="work", bufs=1) as work:
            xT_sb = const.tile([128, CCH, R], f32)
            nc.sync.dma_start(xT_sb, xT)
            wq_sb = const.tile([128, CCH, 3 * C], f32)
            for j in range(6):
                sl = slice(j * 256, (j + 1) * 256)
                nc.sync.dma_start(wq_sb[:, :, sl], wqkvT[:, :, sl])
            wp_sb = const.tile([128, CCH, C], f32)
            nc.sync.dma_start(wp_sb, wprojT)
            pb_sb = const.tile([1, C], f32)
            nc.sync.dma_start(pb_sb, pb)
            corr_sb = const.tile([128, 4, 2], f32)
            nc.sync.dma_start(corr_sb, corr)

            ident = const.tile([128, 128], f32)
            make_identity(nc, ident)
            ones1 = const.tile([1, 128], f32)
            nc.vector.memset(ones1, 1.0)

            # Band masks in transposed [key, head-slot, query] layout.
            # Chunk-1 keys j in [0,128) vs queries i in [0,128): in-band iff
            # 0 <= j - i <= 16.  Chunk-2 keys j2 in [0,16) (global j = 128+j2)
            # vs queries i2 in [0,32) (i = 96+i2): in-band iff j2-i2 in [-32,-16].
            # (walrus codegen only implements is_ge here, so upper bounds are
            # written with negated iota coefficients)
            m17c1 = const.tile([128, 4, 128], f32)
            nc.gpsimd.memset(m17c1, 0.0)
            nc.gpsimd.affine_select(out=m17c1, in_=m17c1, compare_op=is_ge, fill=MASKV,
                                    base=0, channel_multiplier=1,
                                    pattern=[[0, 4], [-1, 128]])
            nc.gpsimd.affine_select(out=m17c1, in_=m17c1, compare_op=is_ge, fill=MASKV,
                                    base=16, channel_multiplier=-1,
                                    pattern=[[0, 4], [1, 128]])
            # inner window (width 9): multiplicative, in-band iff 4 <= j-i <= 12
            m9c1 = const.tile([128, 4, 128], f32)
            nc.gpsimd.memset(m9c1, 1.0)
            nc.gpsimd.affine_select(out=m9c1, in_=m9c1, compare_op=is_ge, fill=0.0,
                                    base=-4, channel_multiplier=1,
                                    pattern=[[0, 4], [-1, 128]])
            nc.gpsimd.affine_select(out=m9c1, in_=m9c1, compare_op=is_ge, fill=0.0,
                                    base=12, channel_multiplier=-1,
                                    pattern=[[0, 4], [1, 128]])
            # Chunk-2 uses the full 128-query width (i in [0,128), keys
            # j = 128 + j2): in-band iff 0 <= (128+j2) - i <= 16.
            m17c2 = const.tile([16, 4, 128], f32)
            nc.gpsimd.memset(m17c2, 0.0)
            nc.gpsimd.affine_select(out=m17c2, in_=m17c2, compare_op=is_ge, fill=MASKV,
                                    base=-112, channel_multiplier=-1,
                                    pattern=[[0, 4], [1, 128]])
            m9c2 = const.tile([16, 4, 128], f32)
            nc.gpsimd.memset(m9c2, 1.0)
            nc.gpsimd.affine_select(out=m9c2, in_=m9c2, compare_op=is_ge, fill=0.0,
                                    base=124, channel_multiplier=1,
                                    pattern=[[0, 4], [-1, 128]])
            nc.gpsimd.affine_select(out=m9c2, in_=m9c2, compare_op=is_ge, fill=0.0,
                                    base=-116, channel_multiplier=-1,
                                    pattern=[[0, 4], [1, 128]])

            # feature-major q|k (fc 0..3 = q heads, 4..7 = k heads), token-major v
            qkT_sb = work.tile([128, 8, R], f32)
            v_sb = work.tile([128, 5, H, D + 1], f32)
            yT_sb = work.tile([128, CCH, LOC], f32)
            nc.vector.memset(v_sb[:, :, :, D:D + 1], 1.0)

            # ---------------- phase 1: qkv projection ----------------
            with tc.tile_pool(name="ps1", bufs=2, space="PSUM") as ps1:
                RC = R // 2  # 264
                for fc in range(8):
                    for rc in range(2):
                        qk_ps = ps1.tile([128, RC], f32, tag="qk")
                        for cc in range(CCH):
                            nc.tensor.matmul(
                                qk_ps,
                                wq_sb[:, cc, fc * 128:(fc + 1) * 128],
                                xT_sb[:, cc, rc * RC:(rc + 1) * RC],
                                start=(cc == 0), stop=(cc == CCH - 1))
                        dst = qkT_sb[:, fc, rc * RC:(rc + 1) * RC]
                        if fc % 2 == 0:
                            nc.vector.tensor_copy(dst, qk_ps)
                        else:
                            nc.scalar.copy(dst, qk_ps)
                for rt in range(5):
                    rsz = 128 if rt < 4 else R - 512
                    v_ps = ps1.tile([128, C], f32, tag="v")
                    for cc in range(CCH):
                        nc.tensor.matmul(
                            v_ps[:rsz],
                            xT_sb[:, cc, rt * 128:rt * 128 + rsz],
                            wq_sb[:, cc, 2 * C:3 * C],
                            start=(cc == 0), stop=(cc == CCH - 1))
                    nc.vector.tensor_copy(
                        v_sb[0:rsz, rt, :, 0:D],
                        v_ps[:rsz].rearrange("p (h d) -> p h d", h=H))

            # ---------------- phase 2: banded attention ----------------
            with tc.tile_pool(name="att", bufs=2) as att, \
                 tc.tile_pool(name="ps_s", bufs=2, space="PSUM") as ps_s, \
                 tc.tile_pool(name="ps_s2", bufs=2, space="PSUM") as ps_s2, \
                 tc.tile_pool(name="ps_pv", bufs=1, space="PSUM") as ps_pv, \
                 tc.tile_pool(name="ps_t", bufs=1, space="PSUM") as ps_t:
                for qb in range(4):
                    y_sb = att.tile([128, H, D], f32, tag="ysb")
                    for g in range(2):
                        s_ps = ps_s.tile([128, 4, 128], f32, tag="s")
                        s2_ps = ps_s2.tile([16, 4, 128], f32, tag="s2")
                        nc.tensor.matmul(s_ps, ident, m17c1,
                                         start=True, stop=False, skip_group_check=True)
                        nc.tensor.matmul(s2_ps, ident[0:16, 0:16], m17c2,
                                         start=True, stop=False, skip_group_check=True)
                        for hh in range(4):
                            h = 4 * g + hh
                            fq, pq = h // 2, (h % 2) * 64
                            kc = 4 + h // 2
                            q1 = qkT_sb[pq:pq + 64, fq, qb * 128 + 8:qb * 128 + 136]
                            k1 = qkT_sb[pq:pq + 64, kc, qb * 128:qb * 128 + 128]
                            k2 = qkT_sb[pq:pq + 64, kc, qb * 128 + 128:qb * 128 + 144]
                            nc.tensor.matmul(s_ps[:, hh, :], k1, q1,
                                             start=False, stop=True, skip_group_check=True)
                            nc.tensor.matmul(s2_ps[:, hh, :], k2, q1,
                                             start=False, stop=True, skip_group_check=True)
                        e17 = att.tile([128, 4, 128], f32, tag="e17")
                        e172 = att.tile([16, 4, 128], f32, tag="e172")
                        nc.scalar.activation(e17, s_ps, Exp)
                        nc.scalar.activation(e172, s2_ps, Exp)
                        e9 = att.tile([128, 4, 128], f32, tag="e9")
                        e92 = att.tile([16, 4, 128], f32, tag="e92")
                        nc.vector.tensor_mul(e9, e17, m9c1)
                        nc.vector.tensor_mul(e92, e172, m9c2)

                        p17 = ps_pv.tile([128, 4, D + 1], f32, tag="pv17")
                        p9 = ps_pv.tile([128, 4, D + 1], f32, tag="pv9")
                        for hh in range(4):
                            h = 4 * g + hh
                            v1 = v_sb[:, qb, h, :]
                            v2 = v_sb[0:16, qb + 1, h, :]
                            nc.tensor.matmul(p17[:, hh, :], e17[:, hh, :], v1,
                                             start=True, stop=False, skip_group_check=True)
                            nc.tensor.matmul(p17[:, hh, :], e172[:, hh, :], v2,
                                             start=False, stop=True, skip_group_check=True)
                            nc.tensor.matmul(p9[:, hh, :], e9[:, hh, :], v1,
                                             start=True, stop=False, skip_group_check=True)
                            nc.tensor.matmul(p9[:, hh, :], e92[:, hh, :], v2,
                                             start=False, stop=True, skip_group_check=True)

                        # denominators sit in column D; fix up batch-edge padding
                        c17 = corr_sb[:, qb:qb + 1, 0:1].to_broadcast([128, 4, 1])
                        c9 = corr_sb[:, qb:qb + 1, 1:2].to_broadcast([128, 4, 1])
                        nc.vector.tensor_add(p17[:, :, D:D + 1], p17[:, :, D:D + 1], c17)
                        nc.vector.tensor_add(p9[:, :, D:D + 1], p9[:, :, D:D + 1], c9)
                        r17 = att.tile([128, 4, 1], f32, tag="r17")
                        r9 = att.tile([128, 4, 1], f32, tag="r9")
                        nc.vector.reciprocal(r17, p17[:, :, D:D + 1])
                        nc.vector.reciprocal(r9, p9[:, :, D:D + 1])
                        t17 = att.tile([128, 4, D], f32, tag="t17")
                        yg = y_sb[:, 4 * g:4 * g + 4, :]
                        nc.vector.tensor_mul(t17, p17[:, :, 0:D],
                                             r17.to_broadcast([128, 4, D]))
                        nc.vector.tensor_mul(yg, p9[:, :, 0:D],
                                             r9.to_broadcast([128, 4, D]))
                        nc.vector.tensor_add(yg, yg, t17)

                    tp = ps_t.tile([128, CCH, 128], f32, tag="tp")
                    for cc in range(CCH):
                        nc.tensor.transpose(
                            tp[:, cc, :],
                            y_sb[:, 2 * cc:2 * cc + 2, :].rearrange("p a b -> p (a b)"),
                            ident)
                    nc.scalar.copy(yT_sb[:, :, qb * 128:(qb + 1) * 128], tp)

            # ---------------- phase 3: output projection ----------------
            with tc.tile_pool(name="ps3", bufs=2, space="PSUM") as ps3, \
                 tc.tile_pool(name="att3", bufs=2) as att3:
                for rt in range(4):
                    po = ps3.tile([128, C], f32, tag="po")
                    nc.tensor.matmul(po, ones1, pb_sb, start=True, stop=False)
                    for cc in range(CCH):
                        nc.tensor.matmul(po, yT_sb[:, cc, rt * 128:(rt + 1) * 128],
                                         wp_sb[:, cc, :],
                                         start=False, stop=(cc == CCH - 1))
                    o_sb = att3.tile([128, C], f32, tag="osb")
                    nc.vector.tensor_copy(o_sb, po)
                    nc.sync.dma_start(y[rt * 128:(rt + 1) * 128, :], o_sb)
    return nc


def _feat_major(a):
    """[C, F] -> [128, C//128, F] with the contraction dim split (cc, p)."""
    c, f = a.shape
    return np.ascontiguousarray(
        a.reshape(c // 128, 128, f).transpose(1, 0, 2)).astype(np.float32)


def prep_in_maps(x, qkv_w, proj_w, proj_b):
    x = np.asarray(x, np.float32)
    qkv_w = np.asarray(qkv_w, np.float32)
    proj_w = np.asarray(proj_w, np.float32)
    proj_b = np.asarray(proj_b, np.float32)

    wq = qkv_w.copy()
    wq[:C] *= SCALE                      # fold 1/sqrt(D) into the q projection
    wqkvT_in = _feat_major(wq.T)         # [128, 4, 1536]
    wprojT_in = _feat_major((proj_w * 0.5).T)  # fold the 2-window average
    pb_in = proj_b.reshape(1, C)

    shards_per_batch = NCORES // B       # 4 shards of 512 tokens per batch row
    in_maps = []
    for s in range(NCORES):
        b, q0 = s // shards_per_batch, (s % shards_per_batch) * LOC
        halo = np.zeros((R, C), np.float32)
        lo, hi = max(0, q0 - HALO), min(N, q0 + LOC + HALO)
        halo[lo - (q0 - HALO):hi - (q0 - HALO)] = x[b, lo:hi]
        xT_in = _feat_major(halo.T)      # [128, 4, 528]

        n = q0 + (np.arange(4)[None, :] * 128 + np.arange(128)[:, None])
        c17 = -(np.maximum(0, HALO - n) + np.maximum(0, n + HALO - (N - 1)))
        c9 = -(np.maximum(0, HALO // 2 - n)
               + np.maximum(0, n + HALO // 2 - (N - 1)))
        corr_in = np.stack([c17, c9], axis=-1).astype(np.float32)  # [128, 4, 2]

        in_maps.append({
            "xT": xT_in,
            "wqkvT": wqkvT_in,
            "wprojT": wprojT_in,
            "pb": pb_in,
            "corr": corr_in,
        })
    return in_maps


def kernel(x, qkv_w, proj_w, proj_b):
    global LAST_RESULTS
    import concourse.mybir as mybir
    from concourse.bass_utils import run_bass_kernel_spmd

    nc = build_module()
    _split_multi_waits(nc, mybir)   # HW-only legalization (CoreSim can't run it)
    in_maps = prep_in_maps(x, qkv_w, proj_w, proj_b)
    trace = os.environ.get("KERNEL_TRACE", "") == "1"
    res = run_bass_kernel_spmd(nc, in_maps, core_ids=list(range(NCORES)),
                               trace=trace)
    LAST_RESULTS = res
    full = np.concatenate([r["y"] for r in res.results], axis=0)  # [4096, 512]
    return np.ascontiguousarray(full.reshape(B, N, C))
